# revision 53
# baseline (speedup 1.0000x reference)
"""Distributed GQA attention kernel for 8 TRN2 NeuronCores.

Problem: B=1, S=2048, D=4096, H=32 q-heads, KV=8 kv-heads, HD=128.
  q = rope(x@wq.T), k = rope(x@wk.T), v = x@wv.T
  out = softmax(causal(q@k.T/sqrt(HD))) @ v @ wo.T

Sharding: tensor-parallel over heads. Core c owns q-heads 4c..4c+3 and
kv-head c. Device-side per core:
  phase 1: QT/KT/VT projections in k-major order (6 PSUM accumulators,
           so the first matmul only needs the first k-slice of x/w —
           smooth DMA ramp). RoPE runs on the Vector engine: the q/k
           head dims are host-permuted even/odd -> halves, making the
           rotation two partition-offset mul-adds instead of PE matmuls.
  phase 2: causal attention, one exp per head-pair, diag mask applied
           by an identity-matmul accumulation on PE (keeps the
           scores->exp chain off the Vector engine). Flat software
           pipeline across all (chunk, pair, tile) units.
  phase 3: all out-projections run at the END; their PE work covers the
           AllGather latency of the later chunks. A full-size dummy
           AllGather during phase 1 absorbs collective setup + RDH
           warm-up. Gathered chunks are staged into SBUF with a
           software-pipelined prefetch.
Host side: layout prep (transposes, head-dim deinterleave, bf16 cast,
rope tables) + final concat/transpose of the 8 out.T slices.
"""

import math
import numpy as np
import ml_dtypes

BF = ml_dtypes.bfloat16

B, S, D = 1, 2048, 4096
H, KV, HD = 32, 8, 128
NCORES = 8
HL = H // NCORES            # 4 local q heads
NP = HL // 2                # 2 local head pairs
QW = HL * HD                # 512 local q width
SC = 512                    # s-chunk width
NSC = S // SC               # 4 s-chunks
KD = 32                     # d-dim k-tiles (4096/128)
NT = S // 128               # 16 t-tiles
SCALE = 1.0 / math.sqrt(HD)
NEG = -30000.0


def _build_nc():
    import concourse.bass as bass
    import concourse.mybir as mybir
    from concourse import bacc, tile

    dt = mybir.dt
    nc = bacc.Bacc()

    xt_d = nc.declare_dram_parameter("xt", [D, S], dt.bfloat16, isOutput=False)
    wqt_d = nc.declare_dram_parameter("wqt", [D, QW], dt.bfloat16, isOutput=False)
    wkt_d = nc.declare_dram_parameter("wkt", [D, HD], dt.bfloat16, isOutput=False)
    wvt_d = nc.declare_dram_parameter("wvt", [D, HD], dt.bfloat16, isOutput=False)
    wot_d = nc.declare_dram_parameter("wot", [D, QW], dt.bfloat16, isOutput=False)
    cosd_d = nc.declare_dram_parameter("cosd", [HD, S], dt.bfloat16, isOutput=False)
    sind_d = nc.declare_dram_parameter("sind", [HD, S], dt.bfloat16, isOutput=False)
    dmask_d = nc.declare_dram_parameter("dmask", [128, 128], dt.bfloat16, isOutput=False)
    ident_d = nc.declare_dram_parameter("ident", [HD, HD], dt.bfloat16, isOutput=False)
    onesc_d = nc.declare_dram_parameter("onesc", [128, 1], dt.bfloat16, isOutput=False)
    onesr_d = nc.declare_dram_parameter("onesr", [1, 128], dt.bfloat16, isOutput=False)
    out_d = nc.declare_dram_parameter("out_t", [QW, S], dt.float32, isOutput=True)

    with tile.TileContext(nc) as tc:
        with (
            tc.tile_pool(name="const", bufs=1) as cpool,
            tc.tile_pool(name="qkv", bufs=1) as qkvpool,
            tc.tile_pool(name="att", bufs=1) as attpool,
            tc.tile_pool(name="dram", bufs=1, space="DRAM") as dpool,
        ):
            # ---- small resident constants ----
            cosd = cpool.tile([HD, S], dt.bfloat16)
            sind = cpool.tile([HD, S], dt.bfloat16)
            dmask = cpool.tile([128, 128], dt.bfloat16)
            ident = cpool.tile([HD, HD], dt.bfloat16)
            onesc = cpool.tile([128, 1], dt.bfloat16)
            onesr = cpool.tile([1, 128], dt.bfloat16)

            def load_consts():
                # dispatched AFTER the first x/w pieces: the SP queue is
                # FIFO and none of these are needed before the first rope
                nc.sync.dma_start(cosd[:], cosd_d[:, :])
                nc.sync.dma_start(sind[:], sind_d[:, :])
                nc.sync.dma_start(dmask[:], dmask_d[:, :])
                nc.sync.dma_start(ident[:], ident_d[:, :])
                nc.sync.dma_start(onesc[:], onesc_d[:, :])
                nc.sync.dma_start(onesr[:], onesr_d[:, :])
                # warm up the ACT exp table load before attention needs it
                warm = cpool.tile([1, 1], dt.float32)
                nc.scalar.activation(warm[:], onesr[0:1, 0:1],
                                     mybir.ActivationFunctionType.Exp)

            # ---- persistent activations (head-PAIR layouts) ----
            qt2 = [qkvpool.tile([HD, 2, S], dt.bfloat16, name=f"qt{p}",
                                tag=f"qt{p}") for p in range(NP)]
            kt = qkvpool.tile([HD, S], dt.bfloat16)
            vv = qkvpool.tile([128, NT, HD], dt.bfloat16)   # [t_part, ti, hd]
            att2 = [attpool.tile([HD, 2, S], dt.bfloat16, name=f"att{p}",
                                 tag=f"att{p}") for p in range(NP)]

            xt_r = xt_d[:, :].rearrange("(k p) s -> p k s", p=128)

            # NOTE: no warm-up collective. The ~70us lazy collective setup
            # freezes the DMA system for ~30us, which starves phase 1's x
            # streaming if triggered early; deferred to the first real
            # gather it lands in the DMA-quiet attention phase, where the
            # CC schedule has ~100us of slack.

            # ================= phase 1: projections + rope =================
            with (
                tc.tile_pool(name="w1", bufs=1) as wpool,
                tc.tile_pool(name="xc", bufs=2) as xpool,
                tc.tile_pool(name="p1", bufs=8, space="PSUM") as pp1,
                tc.tile_pool(name="rtmp", bufs=3) as rtpool,
            ):
                wqt = wpool.tile([128, KD, QW], dt.bfloat16)
                wkt = wpool.tile([128, KD, HD], dt.bfloat16)
                wvt = wpool.tile([128, KD, HD], dt.bfloat16)
                vt = wpool.tile([HD, S], dt.bfloat16)
                wqt_r = wqt_d[:, :].rearrange("(k p) n -> p k n", p=128)
                wkt_r = wkt_d[:, :].rearrange("(k p) n -> p k n", p=128)
                wvt_r = wvt_d[:, :].rearrange("(k p) n -> p k n", p=128)
                # interleave the first x-chunk with weight loads in k-order:
                # the k-major matmul loop consumes exactly in arrival order
                xc0 = xpool.tile([128, KD, SC], dt.bfloat16, tag="xc")
                pieces = [(0, 2), (2, 4), (4, 8), (8, 12), (12, 16),
                          (16, 20), (20, 24), (24, 28), (28, 32)]
                for pi, (lo, hi) in enumerate(pieces):
                    ksl = slice(lo, hi)
                    nc.sync.dma_start(xc0[:, ksl, :], xt_r[:, ksl, 0:SC])
                    nc.sync.dma_start(wqt[:, ksl, :], wqt_r[:, ksl, :])
                    if pi == 1:
                        load_consts()
                # k/v weights arrive while the q-only first pass runs
                for lo, hi in ((0, 16), (16, 32)):
                    ksl = slice(lo, hi)
                    nc.sync.dma_start(wkt[:, ksl, :], wkt_r[:, ksl, :])
                    nc.sync.dma_start(wvt[:, ksl, :], wvt_r[:, ksl, :])

                def rope_evac(ps, dst_r, dst_i, ssl):
                    # deinterleaved rope: rows 0:64 = real r, 64:128 = imag m
                    # out_r = r*c - m*s ; out_m = r*s + m*c. The cross terms
                    # land partition-swapped directly (TT ops allow a
                    # shifted output base when input bases align).
                    qc = rtpool.tile([128, SC], dt.bfloat16, tag="ropeqc")
                    qsw = rtpool.tile([128, SC], dt.bfloat16, tag="ropeqsw")
                    nc.vector.tensor_mul(qc[:], ps[:], cosd[:, ssl])
                    nc.vector.tensor_mul(qsw[0:64, :], ps[64:128, :],
                                         sind[64:128, ssl])
                    nc.vector.tensor_mul(qsw[64:128, :], ps[0:64, :],
                                         sind[0:64, ssl])
                    nc.vector.tensor_sub(dst_r, qc[0:64, :], qsw[0:64, :])
                    nc.vector.tensor_add(dst_i, qc[64:128, :], qsw[64:128, :])

                for sc in range(NSC):
                    ssl = slice(sc * SC, (sc + 1) * SC)
                    if sc == 0:
                        xc = xc0
                    else:
                        xc = xpool.tile([128, KD, SC], dt.bfloat16, tag="xc")
                        for kg in range(4):
                            ksl = slice(kg * 8, (kg + 1) * 8)
                            nc.sync.dma_start(xc[:, ksl, :], xt_r[:, ksl, ssl])

                    def evac(hi, ps):
                        if hi < HL:
                            q2 = qt2[hi // 2]
                            rope_evac(ps, q2[0:64, hi % 2, ssl],
                                      q2[64:128, hi % 2, ssl], ssl)
                        elif hi == HL:
                            rope_evac(ps, kt[0:64, ssl], kt[64:128, ssl], ssl)
                        else:
                            nc.scalar.copy(vt[:, ssl], ps[:])

                    def lhs_of(hi, k):
                        if hi < HL:
                            return wqt[:, k, hi * HD:(hi + 1) * HD]
                        if hi == HL:
                            return wkt[:, k, :]
                        return wvt[:, k, :]

                    if sc == 0:
                        # k-major q-only first pass: matmuls start as soon
                        # as the first k-slices of x/wq arrive, and the
                        # early DMA ramp only has to carry x+wq (8MB);
                        # wk/wv stream in behind it for the second pass
                        pses = [pp1.tile([128, SC], dt.float32, tag="p1",
                                         name=f"p1_{sc}_{i}")
                                for i in range(HL + 2)]
                        for k in range(KD):
                            for hi in range(HL):
                                nc.tensor.matmul(pses[hi][:], lhs_of(hi, k),
                                                 xc[:, k, :], start=(k == 0),
                                                 stop=(k == KD - 1))
                        for k in range(KD):
                            for hi in (HL, HL + 1):
                                nc.tensor.matmul(pses[hi][:], lhs_of(hi, k),
                                                 xc[:, k, :], start=(k == 0),
                                                 stop=(k == KD - 1))
                        for hi in range(HL + 2):
                            evac(hi, pses[hi])
                    else:
                        # head-major: evacuations pipeline behind the next
                        # head's accumulation chain, so the phase boundary
                        # only exposes the last head's drain
                        for hi in range(HL + 2):
                            ps = pp1.tile([128, SC], dt.float32, tag="p1",
                                          name=f"p1_{sc}_{hi}")
                            for k in range(KD):
                                nc.tensor.matmul(ps[:], lhs_of(hi, k),
                                                 xc[:, k, :], start=(k == 0),
                                                 stop=(k == KD - 1))
                            evac(hi, ps)

                    # V tiles in [t, hd] layout via DMA transpose
                    for vtile in range(4):
                        ti = sc * 4 + vtile
                        nc.sync.dma_start_transpose(
                            vv[:, ti, :], vt[:, ti * 128:(ti + 1) * 128])

            # ============ phase 2+3: attention, allgather, out-proj ============
            with (
                tc.tile_pool(name="wo", bufs=1) as wopool,
                tc.tile_pool(name="agc", bufs=2) as agpool,
                tc.tile_pool(name="p2", bufs=2, space="PSUM") as pp2,
                tc.tile_pool(name="pt", bufs=4) as ptpool,
                tc.tile_pool(name="ep", bufs=2) as eppool,
                tc.tile_pool(name="o3", bufs=4) as opool,
            ):
                wot = wopool.tile([128, KD, QW], dt.bfloat16)
                nc.sync.dma_start(
                    wot[:], wot_d[:, :].rearrange("(k p) n -> p k n", p=128))

                def epilogue_a(sc, p, pv2, rs2):
                    # drain PSUM fast: pv copied to SBUF (bf16) right away so
                    # the single pv slot is free before the next pair needs
                    # it; reciprocal (approx_fast ~18 bits) frees rs
                    pvc = eppool.tile([128, 2, SC], dt.bfloat16, tag="pvc")
                    nc.vector.tensor_copy(pvc[:], pv2[:])
                    rec = eppool.tile([1, 2, SC], dt.float32, tag="rec")
                    nc.vector.reciprocal_approx_fast(rec[:], rs2[:])
                    recb = eppool.tile([1, 2, SC], dt.bfloat16, tag="recb")
                    nc.scalar.copy(recb[:], rec[:])
                    return (sc, p, pvc, recb)

                def epilogue_b(sc, p, pvc, recb):
                    # normalize columns of attnT by 1/rowsum; the rank-1
                    # broadcast rides PE (cheap) well after rec is ready
                    ssl = slice(sc * SC, (sc + 1) * SC)
                    bc = pp2.tile([128, 2, SC], dt.float32, tag="st")
                    for h in range(2):
                        nc.tensor.matmul(bc[:, h, :], onesr[:], recb[:, h, :],
                                         start=True, stop=True)
                    nc.vector.tensor_mul(att2[p][:, :, ssl], pvc[:], bc[:])

                def allgather_chunk(sc):
                    # gather this core's att rows for ALL 4 heads of chunk
                    # sc in one collective (fewer ops amortize the ~10us
                    # fixed cost; the serialized CC stream was pacing the
                    # out-proj tail)
                    ssl = slice(sc * SC, (sc + 1) * SC)
                    ag_in = dpool.tile([HL * HD, SC], dt.bfloat16,
                                       name=f"agi{sc}", tag=f"agi{sc}")
                    ag_out = dpool.tile([NCORES * HL * HD, SC], dt.bfloat16,
                                        name=f"ago{sc}", tag=f"ago{sc}",
                                        addr_space="Shared")
                    for p in range(NP):
                        for h in range(2):
                            nc.sync.dma_start(
                                ag_in[(2 * p + h) * HD:(2 * p + h + 1) * HD, :],
                                att2[p][:, h, ssl])
                    nc.gpsimd.collective_compute(
                        "AllGather",
                        mybir.AluOpType.bypass,
                        replica_groups=[list(range(NCORES))],
                        ins=[ag_in.opt()],
                        outs=[ag_out.opt()],
                    )
                    return ag_out

                def scores_exp(sc, p, ti):
                    # scores matmuls for both heads; causal diag mask is
                    # accumulated on PE via an identity matmul, keeping the
                    # scores->exp chain off the Vector engine
                    d_off = ti * 128 - sc * SC
                    v0 = max(d_off, 0)
                    vsl = slice(v0, SC)
                    qcl = slice(sc * SC + v0, (sc + 1) * SC)
                    diag = d_off >= 0
                    st = pp2.tile([128, 2, SC], dt.float32, tag="st")
                    for h in range(2):
                        nc.tensor.matmul(st[:, h, vsl],
                                         kt[:, ti * 128:(ti + 1) * 128],
                                         qt2[p][:, h, qcl],
                                         start=True, stop=not diag,
                                         skip_group_check=diag)
                    if diag:
                        # one identity-matmul accumulates the causal mask
                        # onto BOTH heads' diag blocks (free-dim broadcast)
                        nc.tensor.matmul(
                            st[:, :, d_off:d_off + 128], ident[:],
                            dmask[:, :].unsqueeze(1).broadcast_to([128, 2, 128]),
                            start=False, stop=True, skip_group_check=True)
                    pt = ptpool.tile([128, 2, SC], dt.bfloat16, tag="pt")
                    nc.scalar.activation(pt[:, :, vsl], st[:, :, vsl],
                                         mybir.ActivationFunctionType.Exp,
                                         scale=SCALE)
                    return pt, v0

                pending_ep = None       # deferred (epilogue_b args, sc, p)
                ag_outs = {}            # sc -> ag_out dram tile
                LOOKAHEAD = 2

                def flush_pending():
                    nonlocal pending_ep
                    if pending_ep is not None:
                        ep, gsc, gp = pending_ep
                        epilogue_b(*ep)
                        if gp == NP - 1:
                            # second pair's epilogue completes the chunk;
                            # gather the whole chunk in one op
                            ag_outs[gsc] = allgather_chunk(gsc)
                        pending_ep = None

                # ---- phase 3 staging helpers (agc prefetched into SBUF) ----
                agc_tiles = {}

                def stage(hsc):
                    # 4 sub-DMAs so out-proj chains can start on the first
                    # 8 m-blocks instead of waiting for the full 4MB
                    ag_r = ag_outs[hsc][:, :].rearrange(
                        "(m q) s -> q m s", q=128)
                    agc = agpool.tile([128, NCORES * HL, SC], dt.bfloat16,
                                      tag="agc")
                    for g in range(4):
                        msl = slice(8 * g, 8 * (g + 1))
                        nc.sync.dma_start(agc[:, msl, :], ag_r[:, msl, :])
                    agc_tiles[hsc] = agc

                # flat pipeline over every (chunk, pair, tile) unit so the
                # PE queue stays deep across pair/chunk boundaries
                units = [(sc, p, ti) for sc in range(NSC) for p in range(NP)
                         for ti in range(sc * 4 + 4)]
                cache = {}
                emitted = 0
                pv2 = rs2 = None
                for idx, (sc, p, ti) in enumerate(units):
                    n_t = sc * 4 + 4
                    # ensure depth-2 prologue; steady-state lookahead is
                    # emitted AFTER the consume step below, so the in-order
                    # PE queue never stalls on a scores slot while ready
                    # pv/rs work sits behind it
                    while emitted <= min(idx + 1, len(units) - 1):
                        u = units[emitted]
                        cache[u] = scores_exp(*u)
                        emitted += 1
                    if ti == 0:
                        pv2 = pp2.tile([128, 2, SC], dt.float32,
                                       name="pv2", tag="pv", bufs=1)
                        rs2 = pp2.tile([1, 2, SC], dt.float32,
                                       name="rs2", tag="rs", bufs=1)
                        if (sc, p) == (NSC - 1, 0):
                            # stage early-chunk gathers into SBUF while the
                            # last chunk's attention still runs
                            stage(0)
                            stage(1)
                    if ti == 1:
                        # previous pair's epilogue + gather ride here, well
                        # after its reciprocal is done
                        flush_pending()
                    pt, v0 = cache.pop((sc, p, ti))
                    vsl = slice(v0, SC)
                    for h in range(2):
                        nc.tensor.matmul(
                            pv2[:, h, vsl], vv[:, ti, :], pt[:, h, vsl],
                            start=(ti == 0), stop=(ti == n_t - 1))
                    for h in range(2):
                        nc.tensor.matmul(
                            rs2[:, h, vsl], onesc[:], pt[:, h, vsl],
                            start=(ti == 0), stop=(ti == n_t - 1))
                    if ti == n_t - 1:
                        ep = epilogue_a(sc, p, pv2, rs2)
                        pending_ep = (ep, sc, p)
                    while emitted <= min(idx + LOOKAHEAD, len(units) - 1):
                        u = units[emitted]
                        cache[u] = scores_exp(*u)
                        emitted += 1

                # ---- phase 3: out-projections, all at the end ----
                # gathered block row m = core*HL + head == wot k-group m
                first = True
                for sc in range(NSC):
                    ssl = slice(sc * SC, (sc + 1) * SC)
                    for oc in range(4):
                        ps = pp2.tile([128, 2, SC], dt.float32, tag="st")
                        agc = agc_tiles[sc]
                        for m in range(NCORES * HL):
                            nc.tensor.matmul(
                                ps[:, 0, :],
                                wot[:, m, oc * 128:(oc + 1) * 128],
                                agc[:, m, :],
                                start=(m == 0), stop=(m == NCORES * HL - 1))
                            if first and m == 12:
                                # the last pair's epilogue + gather were
                                # pending while the chain head (pure PE,
                                # inputs ready long ago) was emitted;
                                # release them mid-chain so the final
                                # collective triggers as early as possible
                                flush_pending()
                                stage(2)
                                first = False
                        ot = opool.tile([128, SC], dt.float32, tag="ot")
                        nc.vector.tensor_copy(ot[:], ps[:, 0, :])
                        nc.sync.dma_start(out_d[oc * 128:(oc + 1) * 128, ssl],
                                          ot[:])
                    if sc == 0:
                        stage(3)
    if not nc.is_finalized():
        nc.finalize()
    return nc


_CACHE = {}


def _get_nc():
    if "nc" not in _CACHE:
        _CACHE["nc"] = _build_nc()
    return _CACHE["nc"]


# even/odd -> halves permutation of a 128-wide head dim (applied host-side
# to wq/wk rows, undone implicitly: scores contract q,k over the same
# permuted dim; v/wo untouched)
_PERM = np.concatenate([np.arange(0, HD, 2), np.arange(1, HD, 2)])


def _prep_in_maps(x, wq, wk, wv, wo, freqs_cos, freqs_sin):
    xt = np.ascontiguousarray(x.reshape(S, D).T).astype(BF)
    # deinterleaved rope tables: rows 0:64 and 64:128 both hold the
    # per-frequency cos/sin rows
    cos_t = np.asarray(freqs_cos, np.float32).T          # [64, S]
    sin_t = np.asarray(freqs_sin, np.float32).T
    cosd = np.concatenate([cos_t, cos_t], axis=0).astype(BF)
    sind = np.concatenate([sin_t, sin_t], axis=0).astype(BF)
    ident = np.eye(HD, dtype=np.float32).astype(BF)
    t_idx = np.arange(128)[:, None]
    s_idx = np.arange(128)[None, :]
    dmask = np.where(s_idx >= t_idx, 0.0, NEG).astype(np.float32).astype(BF)
    onesc = np.ones((128, 1), np.float32).astype(BF)
    onesr = np.ones((1, 128), np.float32).astype(BF)

    wq = np.asarray(wq, np.float32).reshape(H, HD, D)[:, _PERM, :].reshape(H * HD, D)
    wk = np.asarray(wk, np.float32).reshape(KV, HD, D)[:, _PERM, :].reshape(KV * HD, D)
    wv = np.asarray(wv, np.float32)
    wo = np.asarray(wo, np.float32)

    in_maps = []
    for c in range(NCORES):
        qsl = slice(QW * c, QW * (c + 1))
        ksl = slice(HD * c, HD * (c + 1))
        in_maps.append({
            "xt": xt,
            "wqt": np.ascontiguousarray(wq[qsl].T).astype(BF),
            "wkt": np.ascontiguousarray(wk[ksl].T).astype(BF),
            "wvt": np.ascontiguousarray(wv[ksl].T).astype(BF),
            "wot": np.ascontiguousarray(wo[qsl].T).astype(BF),
            "cosd": cosd, "sind": sind, "ident": ident,
            "dmask": dmask, "onesc": onesc, "onesr": onesr,
        })
    return in_maps


def run(inputs, trace=False):
    from concourse.bass_utils import run_bass_kernel_spmd
    nc = _get_nc()
    in_maps = _prep_in_maps(
        inputs["x"], inputs["wq"], inputs["wk"], inputs["wv"], inputs["wo"],
        inputs["freqs_cos"], inputs["freqs_sin"])
    res = run_bass_kernel_spmd(nc, in_maps, core_ids=list(range(NCORES)),
                               trace=trace)
    shards = [np.asarray(res.results[c]["out_t"], np.float32)
              for c in range(NCORES)]
    full = np.concatenate(shards, axis=0)          # [4096, 2048]
    out = np.ascontiguousarray(full.T)[None]       # [1, 2048, 4096]
    return out.astype(np.float32), res


def kernel(**inputs):
    out, _ = run(inputs, trace=False)
    return out


# revision 54
# speedup vs baseline: 1.1418x; 1.1418x over previous
"""Distributed GQA attention kernel for 8 TRN2 NeuronCores.

Problem: B=1, S=2048, D=4096, H=32 q-heads, KV=8 kv-heads, HD=128.
  q = rope(x@wq.T), k = rope(x@wk.T), v = x@wv.T
  out = softmax(causal(q@k.T/sqrt(HD))) @ v @ wo.T

Sharding: tensor-parallel over heads. Core c owns q-heads 4c..4c+3 and
kv-head c. Device-side per core:
  phase 1: QT/KT/VT projections in k-major order (6 PSUM accumulators,
           so the first matmul only needs the first k-slice of x/w —
           smooth DMA ramp). RoPE runs on the Vector engine: the q/k
           head dims are host-permuted even/odd -> halves, making the
           rotation two partition-offset mul-adds instead of PE matmuls.
  phase 2: causal attention, one exp per head-pair, diag mask applied
           by an identity-matmul accumulation on PE (keeps the
           scores->exp chain off the Vector engine). Flat software
           pipeline across all (chunk, pair, tile) units.
  phase 3: all out-projections run at the END; their PE work covers the
           AllGather latency of the later chunks. A full-size dummy
           AllGather during phase 1 absorbs collective setup + RDH
           warm-up. Gathered chunks are staged into SBUF with a
           software-pipelined prefetch.
Host side: layout prep (transposes, head-dim deinterleave, bf16 cast,
rope tables) + final concat/transpose of the 8 out.T slices.
"""

import math
import numpy as np
import ml_dtypes

BF = ml_dtypes.bfloat16

B, S, D = 1, 2048, 4096
H, KV, HD = 32, 8, 128
NCORES = 8
HL = H // NCORES            # 4 local q heads
NP = HL // 2                # 2 local head pairs
QW = HL * HD                # 512 local q width
SC = 512                    # s-chunk width
NSC = S // SC               # 4 s-chunks
KD = 32                     # d-dim k-tiles (4096/128)
NT = S // 128               # 16 t-tiles
SCALE = 1.0 / math.sqrt(HD)
NEG = -30000.0


def _build_nc():
    import concourse.bass as bass
    import concourse.mybir as mybir
    from concourse import bacc, tile

    dt = mybir.dt
    nc = bacc.Bacc()

    xt_d = nc.declare_dram_parameter("xt", [D, S], dt.bfloat16, isOutput=False)
    wqt_d = nc.declare_dram_parameter("wqt", [D, QW], dt.bfloat16, isOutput=False)
    wkt_d = nc.declare_dram_parameter("wkt", [D, HD], dt.bfloat16, isOutput=False)
    wvt_d = nc.declare_dram_parameter("wvt", [D, HD], dt.bfloat16, isOutput=False)
    wot_d = nc.declare_dram_parameter("wot", [D, QW], dt.bfloat16, isOutput=False)
    cosd_d = nc.declare_dram_parameter("cosd", [HD, S], dt.bfloat16, isOutput=False)
    sind_d = nc.declare_dram_parameter("sind", [HD, S], dt.bfloat16, isOutput=False)
    dmask_d = nc.declare_dram_parameter("dmask", [128, 128], dt.bfloat16, isOutput=False)
    ident_d = nc.declare_dram_parameter("ident", [HD, HD], dt.bfloat16, isOutput=False)
    onesc_d = nc.declare_dram_parameter("onesc", [128, 1], dt.bfloat16, isOutput=False)
    onesr_d = nc.declare_dram_parameter("onesr", [1, 128], dt.bfloat16, isOutput=False)
    out_d = nc.declare_dram_parameter("out_t", [QW, S], dt.float32, isOutput=True)

    with tile.TileContext(nc) as tc:
        with (
            tc.tile_pool(name="const", bufs=1) as cpool,
            tc.tile_pool(name="qkv", bufs=1) as qkvpool,
            tc.tile_pool(name="att", bufs=1) as attpool,
            tc.tile_pool(name="dram", bufs=1, space="DRAM") as dpool,
        ):
            # ---- small resident constants ----
            cosd = cpool.tile([HD, S], dt.bfloat16)
            sind = cpool.tile([HD, S], dt.bfloat16)
            dmask = cpool.tile([128, 128], dt.bfloat16)
            ident = cpool.tile([HD, HD], dt.bfloat16)
            onesc = cpool.tile([128, 1], dt.bfloat16)
            onesr = cpool.tile([1, 128], dt.bfloat16)

            def load_consts():
                # dispatched AFTER the first x/w pieces: the SP queue is
                # FIFO and none of these are needed before the first rope
                nc.sync.dma_start(cosd[:], cosd_d[:, :])
                nc.sync.dma_start(sind[:], sind_d[:, :])
                nc.sync.dma_start(dmask[:], dmask_d[:, :])
                nc.sync.dma_start(ident[:], ident_d[:, :])
                nc.sync.dma_start(onesc[:], onesc_d[:, :])
                nc.sync.dma_start(onesr[:], onesr_d[:, :])
                # warm up the ACT exp table load before attention needs it
                warm = cpool.tile([1, 1], dt.float32)
                nc.scalar.activation(warm[:], onesr[0:1, 0:1],
                                     mybir.ActivationFunctionType.Exp)

            # ---- persistent activations (head-PAIR layouts) ----
            qt2 = [qkvpool.tile([HD, 2, S], dt.bfloat16, name=f"qt{p}",
                                tag=f"qt{p}") for p in range(NP)]
            kt = qkvpool.tile([HD, S], dt.bfloat16)
            vv = qkvpool.tile([128, NT, HD], dt.bfloat16)   # [t_part, ti, hd]
            att2 = [attpool.tile([HD, 2, S], dt.bfloat16, name=f"att{p}",
                                 tag=f"att{p}") for p in range(NP)]

            xt_r = xt_d[:, :].rearrange("(k p) s -> p k s", p=128)

            # NOTE: no warm-up collective. The ~70us lazy collective setup
            # freezes the DMA system for ~30us, which starves phase 1's x
            # streaming if triggered early; deferred to the first real
            # gather it lands in the DMA-quiet attention phase, where the
            # CC schedule has ~100us of slack.

            # ================= phase 1: projections + rope =================
            with (
                tc.tile_pool(name="w1", bufs=1) as wpool,
                tc.tile_pool(name="xc", bufs=2) as xpool,
                tc.tile_pool(name="p1", bufs=8, space="PSUM") as pp1,
                tc.tile_pool(name="rtmp", bufs=3) as rtpool,
            ):
                wqt = wpool.tile([128, KD, QW], dt.bfloat16)
                wkt = wpool.tile([128, KD, HD], dt.bfloat16)
                wvt = wpool.tile([128, KD, HD], dt.bfloat16)
                vt = wpool.tile([HD, S], dt.bfloat16)
                wqt_r = wqt_d[:, :].rearrange("(k p) n -> p k n", p=128)
                wkt_r = wkt_d[:, :].rearrange("(k p) n -> p k n", p=128)
                wvt_r = wvt_d[:, :].rearrange("(k p) n -> p k n", p=128)
                # interleave the first x-chunk with weight loads in k-order:
                # the k-major matmul loop consumes exactly in arrival order
                xc0 = xpool.tile([128, KD, SC], dt.bfloat16, tag="xc")
                pieces = [(0, 2), (2, 4), (4, 8), (8, 12), (12, 16),
                          (16, 20), (20, 24), (24, 28), (28, 32)]
                for pi, (lo, hi) in enumerate(pieces):
                    ksl = slice(lo, hi)
                    nc.sync.dma_start(xc0[:, ksl, :], xt_r[:, ksl, 0:SC])
                    nc.sync.dma_start(wqt[:, ksl, :], wqt_r[:, ksl, :])
                    nc.sync.dma_start(wkt[:, ksl, :], wkt_r[:, ksl, :])
                    nc.sync.dma_start(wvt[:, ksl, :], wvt_r[:, ksl, :])
                    if pi == 1:
                        load_consts()

                def rope_evac(ps, dst_r, dst_i, ssl):
                    # deinterleaved rope: rows 0:64 = real r, 64:128 = imag m
                    # out_r = r*c - m*s ; out_m = r*s + m*c. The cross terms
                    # land partition-swapped directly (TT ops allow a
                    # shifted output base when input bases align).
                    qc = rtpool.tile([128, SC], dt.bfloat16, tag="ropeqc")
                    qsw = rtpool.tile([128, SC], dt.bfloat16, tag="ropeqsw")
                    nc.vector.tensor_mul(qc[:], ps[:], cosd[:, ssl])
                    nc.vector.tensor_mul(qsw[0:64, :], ps[64:128, :],
                                         sind[64:128, ssl])
                    nc.vector.tensor_mul(qsw[64:128, :], ps[0:64, :],
                                         sind[0:64, ssl])
                    nc.vector.tensor_sub(dst_r, qc[0:64, :], qsw[0:64, :])
                    nc.vector.tensor_add(dst_i, qc[64:128, :], qsw[64:128, :])

                for sc in range(NSC):
                    ssl = slice(sc * SC, (sc + 1) * SC)
                    if sc == 0:
                        xc = xc0
                    else:
                        xc = xpool.tile([128, KD, SC], dt.bfloat16, tag="xc")
                        for kg in range(4):
                            ksl = slice(kg * 8, (kg + 1) * 8)
                            nc.sync.dma_start(xc[:, ksl, :], xt_r[:, ksl, ssl])

                    def evac(hi, ps):
                        if hi < HL:
                            q2 = qt2[hi // 2]
                            rope_evac(ps, q2[0:64, hi % 2, ssl],
                                      q2[64:128, hi % 2, ssl], ssl)
                        elif hi == HL:
                            rope_evac(ps, kt[0:64, ssl], kt[64:128, ssl], ssl)
                        else:
                            nc.scalar.copy(vt[:, ssl], ps[:])

                    def lhs_of(hi, k):
                        if hi < HL:
                            return wqt[:, k, hi * HD:(hi + 1) * HD]
                        if hi == HL:
                            return wkt[:, k, :]
                        return wvt[:, k, :]

                    if sc == 0:
                        # k-major: 6 accumulators, so matmuls start as soon
                        # as the first k-slices of x/w arrive (smooth ramp)
                        pses = [pp1.tile([128, SC], dt.float32, tag="p1",
                                         name=f"p1_{sc}_{i}")
                                for i in range(HL + 2)]
                        for k in range(KD):
                            for hi in range(HL + 2):
                                nc.tensor.matmul(pses[hi][:], lhs_of(hi, k),
                                                 xc[:, k, :], start=(k == 0),
                                                 stop=(k == KD - 1))
                        for hi in range(HL + 2):
                            evac(hi, pses[hi])
                    else:
                        # head-major: evacuations pipeline behind the next
                        # head's accumulation chain, so the phase boundary
                        # only exposes the last head's drain
                        for hi in range(HL + 2):
                            ps = pp1.tile([128, SC], dt.float32, tag="p1",
                                          name=f"p1_{sc}_{hi}")
                            for k in range(KD):
                                nc.tensor.matmul(ps[:], lhs_of(hi, k),
                                                 xc[:, k, :], start=(k == 0),
                                                 stop=(k == KD - 1))
                            evac(hi, ps)

                    # V tiles in [t, hd] layout via DMA transpose
                    for vtile in range(4):
                        ti = sc * 4 + vtile
                        nc.sync.dma_start_transpose(
                            vv[:, ti, :], vt[:, ti * 128:(ti + 1) * 128])

            # ============ phase 2+3: attention, allgather, out-proj ============
            with (
                tc.tile_pool(name="wo", bufs=1) as wopool,
                tc.tile_pool(name="agc", bufs=2) as agpool,
                tc.tile_pool(name="p2", bufs=2, space="PSUM") as pp2,
                tc.tile_pool(name="pt", bufs=4) as ptpool,
                tc.tile_pool(name="ep", bufs=2) as eppool,
                tc.tile_pool(name="o3", bufs=4) as opool,
            ):
                wot = wopool.tile([128, KD, QW], dt.bfloat16)
                nc.sync.dma_start(
                    wot[:], wot_d[:, :].rearrange("(k p) n -> p k n", p=128))

                def epilogue_a(sc, p, pv2, rs2):
                    # drain PSUM fast: pv copied to SBUF (bf16) right away so
                    # the single pv slot is free before the next pair needs
                    # it; reciprocal (approx_fast ~18 bits) frees rs
                    pvc = eppool.tile([128, 2, SC], dt.bfloat16, tag="pvc")
                    nc.vector.tensor_copy(pvc[:], pv2[:])
                    rec = eppool.tile([1, 2, SC], dt.float32, tag="rec")
                    nc.vector.reciprocal_approx_fast(rec[:], rs2[:])
                    recb = eppool.tile([1, 2, SC], dt.bfloat16, tag="recb")
                    nc.scalar.copy(recb[:], rec[:])
                    return (sc, p, pvc, recb)

                def epilogue_b(sc, p, pvc, recb):
                    # normalize columns of attnT by 1/rowsum; the rank-1
                    # broadcast rides PE (cheap) well after rec is ready
                    ssl = slice(sc * SC, (sc + 1) * SC)
                    bc = pp2.tile([128, 2, SC], dt.float32, tag="st")
                    for h in range(2):
                        nc.tensor.matmul(bc[:, h, :], onesr[:], recb[:, h, :],
                                         start=True, stop=True)
                    nc.vector.tensor_mul(att2[p][:, :, ssl], pvc[:], bc[:])

                def allgather_chunk(sc):
                    # gather this core's att rows for ALL 4 heads of chunk
                    # sc in one collective (fewer ops amortize the ~10us
                    # fixed cost; the serialized CC stream was pacing the
                    # out-proj tail)
                    ssl = slice(sc * SC, (sc + 1) * SC)
                    ag_in = dpool.tile([HL * HD, SC], dt.bfloat16,
                                       name=f"agi{sc}", tag=f"agi{sc}")
                    ag_out = dpool.tile([NCORES * HL * HD, SC], dt.bfloat16,
                                        name=f"ago{sc}", tag=f"ago{sc}",
                                        addr_space="Shared")
                    for p in range(NP):
                        for h in range(2):
                            nc.sync.dma_start(
                                ag_in[(2 * p + h) * HD:(2 * p + h + 1) * HD, :],
                                att2[p][:, h, ssl])
                    nc.gpsimd.collective_compute(
                        "AllGather",
                        mybir.AluOpType.bypass,
                        replica_groups=[list(range(NCORES))],
                        ins=[ag_in.opt()],
                        outs=[ag_out.opt()],
                    )
                    return ag_out

                def scores_exp(sc, p, ti):
                    # scores matmuls for both heads; causal diag mask is
                    # accumulated on PE via an identity matmul, keeping the
                    # scores->exp chain off the Vector engine
                    d_off = ti * 128 - sc * SC
                    v0 = max(d_off, 0)
                    vsl = slice(v0, SC)
                    qcl = slice(sc * SC + v0, (sc + 1) * SC)
                    diag = d_off >= 0
                    st = pp2.tile([128, 2, SC], dt.float32, tag="st")
                    for h in range(2):
                        nc.tensor.matmul(st[:, h, vsl],
                                         kt[:, ti * 128:(ti + 1) * 128],
                                         qt2[p][:, h, qcl],
                                         start=True, stop=not diag,
                                         skip_group_check=diag)
                    if diag:
                        # one identity-matmul accumulates the causal mask
                        # onto BOTH heads' diag blocks (free-dim broadcast)
                        nc.tensor.matmul(
                            st[:, :, d_off:d_off + 128], ident[:],
                            dmask[:, :].unsqueeze(1).broadcast_to([128, 2, 128]),
                            start=False, stop=True, skip_group_check=True)
                    pt = ptpool.tile([128, 2, SC], dt.bfloat16, tag="pt")
                    nc.scalar.activation(pt[:, :, vsl], st[:, :, vsl],
                                         mybir.ActivationFunctionType.Exp,
                                         scale=SCALE)
                    return pt, v0

                pending_ep = None       # deferred (epilogue_b args, sc, p)
                ag_outs = {}            # sc -> ag_out dram tile
                LOOKAHEAD = 2

                def flush_pending():
                    nonlocal pending_ep
                    if pending_ep is not None:
                        ep, gsc, gp = pending_ep
                        epilogue_b(*ep)
                        if gp == NP - 1:
                            # second pair's epilogue completes the chunk;
                            # gather the whole chunk in one op
                            ag_outs[gsc] = allgather_chunk(gsc)
                        pending_ep = None

                # ---- phase 3 staging helpers (agc prefetched into SBUF) ----
                agc_tiles = {}

                def stage(hsc):
                    # 4 sub-DMAs so out-proj chains can start on the first
                    # 8 m-blocks instead of waiting for the full 4MB
                    ag_r = ag_outs[hsc][:, :].rearrange(
                        "(m q) s -> q m s", q=128)
                    agc = agpool.tile([128, NCORES * HL, SC], dt.bfloat16,
                                      tag="agc")
                    for g in range(4):
                        msl = slice(8 * g, 8 * (g + 1))
                        nc.sync.dma_start(agc[:, msl, :], ag_r[:, msl, :])
                    agc_tiles[hsc] = agc

                # flat pipeline over every (chunk, pair, tile) unit so the
                # PE queue stays deep across pair/chunk boundaries
                units = [(sc, p, ti) for sc in range(NSC) for p in range(NP)
                         for ti in range(sc * 4 + 4)]
                cache = {}
                emitted = 0
                pv2 = rs2 = None
                for idx, (sc, p, ti) in enumerate(units):
                    n_t = sc * 4 + 4
                    # ensure depth-2 prologue; steady-state lookahead is
                    # emitted AFTER the consume step below, so the in-order
                    # PE queue never stalls on a scores slot while ready
                    # pv/rs work sits behind it
                    while emitted <= min(idx + 1, len(units) - 1):
                        u = units[emitted]
                        cache[u] = scores_exp(*u)
                        emitted += 1
                    if ti == 0:
                        pv2 = pp2.tile([128, 2, SC], dt.float32,
                                       name="pv2", tag="pv", bufs=1)
                        rs2 = pp2.tile([1, 2, SC], dt.float32,
                                       name="rs2", tag="rs", bufs=1)
                        if (sc, p) == (NSC - 1, 0):
                            # stage early-chunk gathers into SBUF while the
                            # last chunk's attention still runs
                            stage(0)
                            stage(1)
                    if ti == 1:
                        # previous pair's epilogue + gather ride here, well
                        # after its reciprocal is done
                        flush_pending()
                    pt, v0 = cache.pop((sc, p, ti))
                    vsl = slice(v0, SC)
                    for h in range(2):
                        nc.tensor.matmul(
                            pv2[:, h, vsl], vv[:, ti, :], pt[:, h, vsl],
                            start=(ti == 0), stop=(ti == n_t - 1))
                    for h in range(2):
                        nc.tensor.matmul(
                            rs2[:, h, vsl], onesc[:], pt[:, h, vsl],
                            start=(ti == 0), stop=(ti == n_t - 1))
                    if ti == n_t - 1:
                        ep = epilogue_a(sc, p, pv2, rs2)
                        pending_ep = (ep, sc, p)
                    while emitted <= min(idx + LOOKAHEAD, len(units) - 1):
                        u = units[emitted]
                        cache[u] = scores_exp(*u)
                        emitted += 1

                # ---- phase 3: out-projections, all at the end ----
                # gathered block row m = core*HL + head == wot k-group m
                first = True
                for sc in range(NSC):
                    ssl = slice(sc * SC, (sc + 1) * SC)
                    for oc in range(4):
                        ps = pp2.tile([128, 2, SC], dt.float32, tag="st")
                        agc = agc_tiles[sc]
                        for m in range(NCORES * HL):
                            nc.tensor.matmul(
                                ps[:, 0, :],
                                wot[:, m, oc * 128:(oc + 1) * 128],
                                agc[:, m, :],
                                start=(m == 0), stop=(m == NCORES * HL - 1))
                            if first and m == 12:
                                # the last pair's epilogue + gather were
                                # pending while the chain head (pure PE,
                                # inputs ready long ago) was emitted;
                                # release them mid-chain so the final
                                # collective triggers as early as possible
                                flush_pending()
                                stage(2)
                                first = False
                        ot = opool.tile([128, SC], dt.float32, tag="ot")
                        nc.vector.tensor_copy(ot[:], ps[:, 0, :])
                        nc.sync.dma_start(out_d[oc * 128:(oc + 1) * 128, ssl],
                                          ot[:])
                    if sc == 0:
                        stage(3)
    if not nc.is_finalized():
        nc.finalize()
    return nc


_CACHE = {}


def _get_nc():
    if "nc" not in _CACHE:
        _CACHE["nc"] = _build_nc()
    return _CACHE["nc"]


# even/odd -> halves permutation of a 128-wide head dim (applied host-side
# to wq/wk rows, undone implicitly: scores contract q,k over the same
# permuted dim; v/wo untouched)
_PERM = np.concatenate([np.arange(0, HD, 2), np.arange(1, HD, 2)])


def _prep_in_maps(x, wq, wk, wv, wo, freqs_cos, freqs_sin):
    xt = np.ascontiguousarray(x.reshape(S, D).T).astype(BF)
    # deinterleaved rope tables: rows 0:64 and 64:128 both hold the
    # per-frequency cos/sin rows
    cos_t = np.asarray(freqs_cos, np.float32).T          # [64, S]
    sin_t = np.asarray(freqs_sin, np.float32).T
    cosd = np.concatenate([cos_t, cos_t], axis=0).astype(BF)
    sind = np.concatenate([sin_t, sin_t], axis=0).astype(BF)
    ident = np.eye(HD, dtype=np.float32).astype(BF)
    t_idx = np.arange(128)[:, None]
    s_idx = np.arange(128)[None, :]
    dmask = np.where(s_idx >= t_idx, 0.0, NEG).astype(np.float32).astype(BF)
    onesc = np.ones((128, 1), np.float32).astype(BF)
    onesr = np.ones((1, 128), np.float32).astype(BF)

    wq = np.asarray(wq, np.float32).reshape(H, HD, D)[:, _PERM, :].reshape(H * HD, D)
    wk = np.asarray(wk, np.float32).reshape(KV, HD, D)[:, _PERM, :].reshape(KV * HD, D)
    wv = np.asarray(wv, np.float32)
    wo = np.asarray(wo, np.float32)

    in_maps = []
    for c in range(NCORES):
        qsl = slice(QW * c, QW * (c + 1))
        ksl = slice(HD * c, HD * (c + 1))
        in_maps.append({
            "xt": xt,
            "wqt": np.ascontiguousarray(wq[qsl].T).astype(BF),
            "wkt": np.ascontiguousarray(wk[ksl].T).astype(BF),
            "wvt": np.ascontiguousarray(wv[ksl].T).astype(BF),
            "wot": np.ascontiguousarray(wo[qsl].T).astype(BF),
            "cosd": cosd, "sind": sind, "ident": ident,
            "dmask": dmask, "onesc": onesc, "onesr": onesr,
        })
    return in_maps


def run(inputs, trace=False):
    from concourse.bass_utils import run_bass_kernel_spmd
    nc = _get_nc()
    in_maps = _prep_in_maps(
        inputs["x"], inputs["wq"], inputs["wk"], inputs["wv"], inputs["wo"],
        inputs["freqs_cos"], inputs["freqs_sin"])
    res = run_bass_kernel_spmd(nc, in_maps, core_ids=list(range(NCORES)),
                               trace=trace)
    shards = [np.asarray(res.results[c]["out_t"], np.float32)
              for c in range(NCORES)]
    full = np.concatenate(shards, axis=0)          # [4096, 2048]
    out = np.ascontiguousarray(full.T)[None]       # [1, 2048, 4096]
    return out.astype(np.float32), res


def kernel(**inputs):
    out, _ = run(inputs, trace=False)
    return out
